# revision 4
# baseline (speedup 1.0000x reference)
"""2-layer GCN (GCNConv x2, relu) on 8 Trainium2 NeuronCores.

Strategy (dest-shard, aggregate in 128-dim space):
  out1 = relu((A x) @ W1 + b1)            [A@(x@W1) == (A@x)@W1]
  zg   = out1 @ W2, allgathered
  out2 = relu(A zg + b2)
A = Dc(Mw + I)Dr; the full norm dinv[src]*w*dinv[dst] is folded into the
one-hot edge weights on host (dinv computed host-side from edge_weight).

Layer 1 needs no device gather: the host ships x rows pre-arranged in
edge-chunk order (xe[slot, chunk, feat]); chunks stream sequentially.
Layer 2 gathers zg rows by edge via gpsimd dma_gather (int16 idx, source
split in 2 halves < 32768 rows). Self-loop terms in layer 2 avoid the
gather: they read the local zg_shard slab sequentially and use a
host-built diagonal one-hot.
Per chunk: one-hot oh[e,d] = (iota==lc_e)*w_e via one DVE tensor_scalar,
matmul-accumulate into the dest panel's PSUM tile (L1 feat-major,
L2 dest-major).
"""
import sys
import numpy as np

sys.path.insert(0, "/opt/trn_rl_repo")

import concourse.bass as bass  # noqa: F401
import concourse.bacc as bacc
import concourse.mybir as mybir
import concourse.tile as tile
from concourse.bass_utils import run_bass_kernel_spmd
from concourse.masks import make_identity

P = 128
NCORES = 8
G = 32  # chunks per wave (both load and gather waves)

F32 = mybir.dt.float32
F16 = mybir.dt.float16
I16 = mybir.dt.int16
MSG_DT = F16


# ---------------------------------------------------------------- CPU prep


def _pack_idx(idx_flat):
    """int16 indices -> [128, ceil(n/16)] wrapped + 8x replicated layout."""
    n = len(idx_flat)
    n16 = -(-n // 16)
    buf = np.zeros(16 * n16, dtype=np.int16)
    buf[:n] = idx_flat
    blk = buf.reshape(n16, 16).T  # idx j at [j%16, j//16]
    return np.tile(blk, (8, 1)).copy()


def preprocess(x, edge_index, edge_weight, n):
    """Per-core inputs + universal chunk grids.

    Dests grouped into panels of <=128 CONSECUTIVE dests, boundaries per
    core; per-panel chunk counts are universal (max over cores).
    """
    row = np.asarray(edge_index[0], dtype=np.int64)
    col = np.asarray(edge_index[1], dtype=np.int64)
    w = np.asarray(edge_weight, dtype=np.float32)
    shard = n // NCORES
    half = (n + 1) // 2

    # host-side gcn_norm (f64): deg includes self loop weight 1
    deg = np.ones(n, np.float64)
    np.add.at(deg, col, w.astype(np.float64))
    dinv = 1.0 / np.sqrt(deg)
    norm = (dinv[row] * w * dinv[col]).astype(np.float32)
    selfw = (dinv * dinv).astype(np.float32)

    core_of = col // shard
    ind_lo = np.bincount(col[row < half], minlength=n)
    ind_hi = np.bincount(col[row >= half], minlength=n)

    # ---- panel boundaries per core (greedy fill, caps per L2 stream)
    CAP = 7 * P
    blist = []
    for k in range(NCORES):
        lo_c = ind_lo[k * shard:(k + 1) * shard]
        hi_c = ind_hi[k * shard:(k + 1) * shard]
        b = [0]
        cl = ch = cd = 0
        for ld in range(shard):
            if cd == P or cl + lo_c[ld] > CAP or ch + hi_c[ld] > CAP:
                b.append(ld)
                cl = ch = cd = 0
            cl += lo_c[ld]
            ch += hi_c[ld]
            cd += 1
        b.append(shard)
        blist.append(b)
    npanel = max(len(b) - 1 for b in blist)
    bounds = np.zeros((NCORES, npanel + 1), np.int64)
    for k in range(NCORES):
        b = blist[k]
        while len(b) < npanel + 1:
            b.append(shard)
        bounds[k] = b

    # ---- per (core, panel) edge counts -> universal grids
    c1 = np.zeros((NCORES, npanel), np.int64)   # L1: edges + selfs
    clo = np.zeros((NCORES, npanel), np.int64)  # L2 lo (no selfs)
    chi = np.zeros((NCORES, npanel), np.int64)
    ind_all = ind_lo + ind_hi
    for k in range(NCORES):
        for j in range(npanel):
            a, b2 = bounds[k, j], bounds[k, j + 1]
            sl = slice(k * shard + a, k * shard + b2)
            c1[k, j] = ind_all[sl].sum() + (b2 - a)
            clo[k, j] = ind_lo[sl].sum()
            chi[k, j] = ind_hi[sl].sum()
    k1 = np.maximum(1, -(-c1.max(axis=0) // P))
    klo = -(-clo.max(axis=0) // P)
    khi = -(-chi.max(axis=0) // P)
    nch1 = int(k1.sum())
    nlo, nhi = int(klo.sum()), int(khi.sum())
    nch2 = npanel + nlo + nhi  # + one self chunk per panel

    chunks1 = []
    for j in range(npanel):
        for i in range(int(k1[j])):
            chunks1.append(dict(panel=j, first=(i == 0),
                                last=(i == int(k1[j]) - 1)))
    chunks2 = []
    lo_pos = hi_pos = 0
    for j in range(npanel):
        nj = 1 + int(klo[j]) + int(khi[j])
        chunks2.append(dict(stream=2, pos=j, panel=j, first=True,
                            last=(nj == 1)))
        ci = 1
        for i in range(int(klo[j])):
            ci += 1
            chunks2.append(dict(stream=0, pos=lo_pos, panel=j, first=False,
                                last=(ci == nj)))
            lo_pos += 1
        for i in range(int(khi[j])):
            ci += 1
            chunks2.append(dict(stream=1, pos=hi_pos, panel=j, first=False,
                                last=(ci == nj)))
            hi_pos += 1

    # ---- AG position map: node -> row in padded zg_full
    agpos = np.zeros(n, np.int64)
    for k in range(NCORES):
        for j in range(npanel):
            a, b2 = bounds[k, j], bounds[k, j + 1]
            if b2 > a:
                agpos[k * shard + a:k * shard + b2] = \
                    k * npanel * P + j * P + np.arange(b2 - a)
    h2 = int(agpos[half - 1]) + 1  # L2 lo/hi split row in zg_full
    assert h2 < 32768 and (NCORES * npanel * P - h2) < 32768

    # ---- per-core slot data
    cores = []
    for k in range(NCORES):
        m = core_of == k
        r_k, c_k, w_k = row[m], col[m], norm[m]
        ld = c_k - k * shard
        panel = np.searchsorted(bounds[k], ld, side="right") - 1
        q = ld - bounds[k][panel]
        hi = (r_k >= half).astype(np.int64)
        order = np.lexsort((hi, panel))
        r_k, w_k, panel, hi, q = (r_k[order], w_k[order], panel[order],
                                  hi[order], q[order])

        # L1 slots: per panel, real edges then selfs
        src1 = np.zeros(nch1 * P, np.int64)
        lc1 = np.zeros((P, nch1), np.float32)
        w1v = np.zeros((P, nch1), np.float32)
        off1 = np.r_[0, np.cumsum(k1)]
        pstart = np.searchsorted(panel, np.arange(npanel))
        pend = np.searchsorted(panel, np.arange(npanel), side="right")
        for j in range(npanel):
            a, b2 = bounds[k, j], bounds[k, j + 1]
            width = int(b2 - a)
            ss, se = int(pstart[j]), int(pend[j])
            srcs = np.concatenate([
                r_k[ss:se],
                np.arange(k * shard + a, k * shard + b2)])
            lcs = np.concatenate([q[ss:se], np.arange(width)])
            wvs = np.concatenate([w_k[ss:se],
                                  selfw[k * shard + a:k * shard + b2]])
            base = int(off1[j]) * P
            ne = len(srcs)
            src1[base:base + ne] = srcs
            fl = lcs.astype(np.float32)
            fv = wvs.astype(np.float32)
            cix = np.arange(ne) // P + int(off1[j])
            eix = np.arange(ne) % P
            lc1[eix, cix] = fl
            w1v[eix, cix] = fv

        # L2 slots: lo/hi streams by (panel, hi) groups
        key = panel * 2 + hi
        cnt = np.bincount(key, minlength=npanel * 2)
        goff = np.r_[0, np.cumsum(cnt)]
        idx_lo = np.zeros(nlo * P, np.int64)
        idx_hi = np.zeros(nhi * P, np.int64)
        lc2 = np.zeros((P, nch2), np.float32)
        w2v = np.zeros((P, nch2), np.float32)
        ag_r = agpos[r_k]
        win = np.zeros(npanel * 2, np.int64)
        lo_i = hi_i = 0
        for ci, c in enumerate(chunks2):
            st, j = c["stream"], c["panel"]
            if st == 2:
                continue
            g2 = j * 2 + st
            a = goff[g2] + win[g2] * P
            b2 = min(goff[g2] + win[g2] * P + P, goff[g2 + 1])
            win[g2] += 1
            m2 = max(0, int(b2 - a))
            if m2 > 0:
                if st == 0:
                    pos = c["pos"] * P
                    idx_lo[pos:pos + m2] = ag_r[a:b2]
                else:
                    pos = c["pos"] * P
                    idx_hi[pos:pos + m2] = ag_r[a:b2] - h2
                lc2[:m2, ci] = q[a:b2]
                w2v[:m2, ci] = w_k[a:b2]

        # self diagonal one-hot [P, npanel, P]
        diag2 = np.zeros((P, npanel, P), np.float16)
        for j in range(npanel):
            a, b2 = bounds[k, j], bounds[k, j + 1]
            width = int(b2 - a)
            qq = np.arange(width)
            diag2[qq, j, qq] = selfw[k * shard + a:k * shard + b2]

        cores.append(dict(
            src1=src1, lc1=lc1, w1v=w1v, lc2=lc2, w2v=w2v,
            idx2_lo=_pack_idx(idx_lo.astype(np.int16)),
            idx2_hi=_pack_idx(idx_hi.astype(np.int16)),
            diag2=diag2, bounds=bounds[k].copy()))

    spec = dict(n=n, shard=shard, npanel=npanel, half=half, h2=h2,
                chunks1=chunks1, chunks2=chunks2, nch1=nch1, nch2=nch2,
                nlo=nlo, nhi=nhi)
    return spec, cores


# ---------------------------------------------------------------- program


def build_program(spec, din, dhid, dout):
    npanel, shard = spec["npanel"], spec["shard"]
    h2 = spec["h2"]
    nrows_pad = npanel * P
    nfull = NCORES * nrows_pad
    chunks1, chunks2 = spec["chunks1"], spec["chunks2"]
    nch1, nch2 = spec["nch1"], spec["nch2"]
    nlo, nhi = spec["nlo"], spec["nhi"]
    assert din == P and dout == P and dhid == 2 * P

    nc = bacc.Bacc("TRN2", target_bir_lowering=False, debug=False,
                   num_devices=NCORES)
    dt = F32
    xe_d = nc.dram_tensor("xe", [P, nch1, din], MSG_DT, kind="ExternalInput")
    w1_d = nc.dram_tensor("w1", [din, dhid], MSG_DT, kind="ExternalInput")
    w2_d = nc.dram_tensor("w2", [dhid, dout], MSG_DT, kind="ExternalInput")
    lc1_d = nc.dram_tensor("lc1", [P, nch1], dt, kind="ExternalInput")
    w1v_d = nc.dram_tensor("w1v", [P, nch1], dt, kind="ExternalInput")
    lc2_d = nc.dram_tensor("lc2", [P, nch2], dt, kind="ExternalInput")
    w2v_d = nc.dram_tensor("w2v", [P, nch2], dt, kind="ExternalInput")
    i2lo_d = nc.dram_tensor("idx2_lo", [P, max(nlo, 1) * 8], I16,
                            kind="ExternalInput")
    i2hi_d = nc.dram_tensor("idx2_hi", [P, max(nhi, 1) * 8], I16,
                            kind="ExternalInput")
    diag2_d = nc.dram_tensor("diag2", [P, npanel, P], MSG_DT,
                             kind="ExternalInput")
    iota_d = nc.dram_tensor("iota", [P, P], dt, kind="ExternalInput")
    out_d = nc.dram_tensor("out", [nrows_pad, dout], dt,
                           kind="ExternalOutput")

    with tile.TileContext(nc) as tc:
        with (
            tc.tile_pool(name="const", bufs=1) as cpool,
            tc.tile_pool(name="dram", bufs=1, space="DRAM") as dram,
        ):
            zg_shard = dram.tile([nrows_pad, dout], MSG_DT)
            zg_full = dram.tile([nfull, dout], MSG_DT)

            iota_sb = cpool.tile([P, P], dt, tag="iota")
            nc.sync.dma_start(out=iota_sb[:], in_=iota_d[:])
            ident = cpool.tile([P, P], MSG_DT, tag="ident")
            make_identity(nc, ident[:])
            w1_sb = cpool.tile([din, dhid], MSG_DT, tag="w1")
            nc.sync.dma_start(out=w1_sb[:], in_=w1_d[:])
            w2a_sb = cpool.tile([P, dout], MSG_DT, tag="w2a")
            nc.sync.dma_start(out=w2a_sb[:], in_=w2_d[0:P, :])
            w2b_sb = cpool.tile([P, dout], MSG_DT, tag="w2b")
            nc.sync.dma_start(out=w2b_sb[:], in_=w2_d[P:2 * P, :])
            lc1_sb = cpool.tile([P, nch1], dt, tag="lc1")
            nc.sync.dma_start(out=lc1_sb[:], in_=lc1_d[:])
            w1v_sb = cpool.tile([P, nch1], dt, tag="w1v")
            nc.sync.dma_start(out=w1v_sb[:], in_=w1v_d[:])
            lc2_sb = cpool.tile([P, nch2], dt, tag="lc2")
            nc.sync.dma_start(out=lc2_sb[:], in_=lc2_d[:])
            w2v_sb = cpool.tile([P, nch2], dt, tag="w2v")
            nc.sync.dma_start(out=w2v_sb[:], in_=w2v_d[:])
            i2lo_sb = cpool.tile([P, max(nlo, 1) * 8], I16, tag="i2lo")
            nc.sync.dma_start(out=i2lo_sb[:], in_=i2lo_d[:])
            i2hi_sb = cpool.tile([P, max(nhi, 1) * 8], I16, tag="i2hi")
            nc.sync.dma_start(out=i2hi_sb[:], in_=i2hi_d[:])
            diag2_sb = cpool.tile([P, npanel, P], MSG_DT, tag="diag2")
            nc.sync.dma_start(out=diag2_sb[:], in_=diag2_d[:])

            # -------- layer 1: stream xe chunks, one-hot matmul ----------
            def l1_panel(j, psum, epp, sbp):
                aggT = sbp.tile([P, P], MSG_DT, tag="aggT")
                nc.vector.tensor_copy(out=aggT[:], in_=psum[:])
                h1p = epp.tile([P, dhid], F32, space="PSUM", tag="h1p")
                nc.tensor.matmul(out=h1p[:], lhsT=aggT[:], rhs=w1_sb[:],
                                 start=True, stop=True)
                h1 = sbp.tile([P, dhid], MSG_DT, tag="h1")
                nc.vector.tensor_scalar(out=h1[:], in0=h1p[:],
                                        scalar1=0.0, scalar2=None,
                                        op0=mybir.AluOpType.max)
                tp0 = epp.tile([P, P], MSG_DT, space="PSUM", tag="tp0")
                nc.tensor.transpose(out=tp0[:], in_=h1[:, 0:P],
                                    identity=ident[:])
                tp1 = epp.tile([P, P], MSG_DT, space="PSUM", tag="tp1")
                nc.tensor.transpose(out=tp1[:], in_=h1[:, P:2 * P],
                                    identity=ident[:])
                h1t0 = sbp.tile([P, P], MSG_DT, tag="h1t0")
                nc.vector.tensor_copy(out=h1t0[:], in_=tp0[:])
                h1t1 = sbp.tile([P, P], MSG_DT, tag="h1t1")
                nc.vector.tensor_copy(out=h1t1[:], in_=tp1[:])
                zp = epp.tile([P, dout], F32, space="PSUM", tag="zp")
                nc.tensor.matmul(out=zp[:], lhsT=h1t0[:], rhs=w2a_sb[:],
                                 start=True, stop=False)
                nc.tensor.matmul(out=zp[:], lhsT=h1t1[:], rhs=w2b_sb[:],
                                 start=False, stop=True)
                zg = sbp.tile([P, dout], MSG_DT, tag="zg")
                nc.vector.tensor_copy(out=zg[:], in_=zp[:])
                nc.sync.dma_start(out=zg_shard[j * P:(j + 1) * P, :],
                                  in_=zg[:])

            with (
                tc.tile_pool(name="xw", bufs=3) as xwp,
                tc.tile_pool(name="oh1", bufs=8) as ohp1,
                tc.tile_pool(name="agg1", bufs=2, space="PSUM") as aggp1,
                tc.tile_pool(name="ep1", bufs=1, space="PSUM") as epp1,
                tc.tile_pool(name="sb1", bufs=3) as sbp1,
            ):
                wave_t = None
                psum = None
                for ci, c in enumerate(chunks1):
                    wv, slot = divmod(ci, G)
                    if slot == 0:
                        gsz = min(G, nch1 - wv * G)
                        wave_t = xwp.tile([P, G, P], MSG_DT, tag="xw")
                        nc.sync.dma_start(
                            out=wave_t[:, :gsz, :],
                            in_=xe_d[:, wv * G:wv * G + gsz, :])
                    gt = wave_t[:, slot, :]
                    oh = ohp1.tile([P, P], MSG_DT, tag="oh")
                    nc.vector.tensor_scalar(
                        out=oh[:], in0=iota_sb[:],
                        scalar1=lc1_sb[:, ci:ci + 1],
                        scalar2=w1v_sb[:, ci:ci + 1],
                        op0=mybir.AluOpType.is_equal,
                        op1=mybir.AluOpType.mult)
                    if c["first"]:
                        psum = aggp1.tile([P, P], F32, space="PSUM",
                                          tag="agg")
                    # feat-major: psum[f,d] += gt.T @ oh
                    nc.tensor.matmul(out=psum[:], lhsT=gt, rhs=oh[:],
                                     start=c["first"], stop=c["last"])
                    if c["last"]:
                        l1_panel(c["panel"], psum, epp1, sbp1)

            # -------- allgather ------------------------------------------
            nc.gpsimd.collective_compute(
                "AllGather", mybir.AluOpType.bypass,
                replica_groups=[list(range(NCORES))],
                ins=[zg_shard.opt()], outs=[zg_full.opt()])

            # -------- layer 2: gather zg, one-hot matmul -----------------
            def l2_panel(j, psum, sbp):
                o = sbp.tile([P, dout], dt, tag="o2")
                nc.vector.tensor_scalar(out=o[:], in0=psum[:],
                                        scalar1=0.0, scalar2=None,
                                        op0=mybir.AluOpType.max)
                nc.sync.dma_start(out=out_d[j * P:(j + 1) * P, :],
                                  in_=o[:])

            with (
                tc.tile_pool(name="glo", bufs=3) as glo,
                tc.tile_pool(name="ghi", bufs=3) as ghi,
                tc.tile_pool(name="gse", bufs=3) as gse,
                tc.tile_pool(name="oh2", bufs=8) as ohp2,
                tc.tile_pool(name="agg2", bufs=4, space="PSUM") as aggp2,
                tc.tile_pool(name="sb2", bufs=3) as sbp2,
            ):
                wave_t = [None, None]
                psum = None
                for ci, c in enumerate(chunks2):
                    st, pos, j = c["stream"], c["pos"], c["panel"]
                    if st == 2:
                        gt_t = gse.tile([P, P], MSG_DT, tag="gse")
                        nc.sync.dma_start(
                            out=gt_t[:],
                            in_=zg_shard[j * P:(j + 1) * P, :])
                        gt = gt_t[:]
                        oh = diag2_sb[:, j, :]
                    else:
                        wv, slot = divmod(pos, G)
                        if slot == 0:
                            pool = glo if st == 0 else ghi
                            idx_sb = i2lo_sb if st == 0 else i2hi_sb
                            nw = nlo if st == 0 else nhi
                            src = (zg_full[0:h2, :] if st == 0
                                   else zg_full[h2:nfull, :])
                            gsz = min(G, nw - wv * G)
                            t = pool.tile([P, G, P], MSG_DT, tag="gw")
                            nc.gpsimd.dma_gather(
                                out_ap=t[:, :gsz, :], in_ap=src,
                                idxs_ap=idx_sb[:, wv * G * 8:
                                               wv * G * 8 + gsz * 8],
                                num_idxs=gsz * P, num_idxs_reg=gsz * P,
                                elem_size=P, single_packet=False)
                            wave_t[st] = t
                        gt = wave_t[st][:, slot, :]
                        oh_t = ohp2.tile([P, P], MSG_DT, tag="oh")
                        nc.vector.tensor_scalar(
                            out=oh_t[:], in0=iota_sb[:],
                            scalar1=lc2_sb[:, ci:ci + 1],
                            scalar2=w2v_sb[:, ci:ci + 1],
                            op0=mybir.AluOpType.is_equal,
                            op1=mybir.AluOpType.mult)
                        oh = oh_t[:]
                    if c["first"]:
                        psum = aggp2.tile([P, P], F32, space="PSUM",
                                          tag="agg")
                    # dest-major: psum[d,f] += oh.T @ gt
                    nc.tensor.matmul(out=psum[:], lhsT=oh, rhs=gt,
                                     start=c["first"], stop=c["last"])
                    if c["last"]:
                        l2_panel(j, psum, sbp2)

    nc.compile()
    return nc


# ---------------------------------------------------------------- kernel


def make_in_maps(spec, cores, x, W1, W2):
    n = spec["n"]
    nch1 = spec["nch1"]
    x32 = np.asarray(x, dtype=np.float32)
    W1m = np.asarray(W1, dtype=np.float32).astype(np.float16)
    W2m = np.asarray(W2, dtype=np.float32).astype(np.float16)
    iota_np = np.tile(np.arange(P, dtype=np.float32), (P, 1))
    in_maps = []
    for k in range(NCORES):
        c = cores[k]
        xe = x32[c["src1"]].astype(np.float16)      # [nch1*P, din]
        mask = c["w1v"].T.reshape(-1) != 0           # zero padded slots
        xe[~mask] = 0
        xe = xe.reshape(nch1, P, -1).transpose(1, 0, 2).copy()
        in_maps.append(dict(
            xe=xe, w1=W1m, w2=W2m, lc1=c["lc1"], w1v=c["w1v"],
            lc2=c["lc2"], w2v=c["w2v"], idx2_lo=c["idx2_lo"],
            idx2_hi=c["idx2_hi"], diag2=c["diag2"], iota=iota_np))
    return in_maps


def kernel(x, edge_index, edge_weight, W1, b1, W2, b2):
    x = np.asarray(x, dtype=np.float32)
    W1 = np.asarray(W1, dtype=np.float32)
    W2 = np.asarray(W2, dtype=np.float32)
    n, din = x.shape
    dhid, dout = W1.shape[1], W2.shape[1]
    assert not np.any(np.asarray(b1)) and not np.any(np.asarray(b2))

    spec, cores = preprocess(x, edge_index, edge_weight, n)
    nc = build_program(spec, din, dhid, dout)
    in_maps = make_in_maps(spec, cores, x, W1, W2)

    res = run_bass_kernel_spmd(nc, in_maps, core_ids=list(range(NCORES)))
    out = np.empty((n, dout), dtype=np.float32)
    npanel = spec["npanel"]
    shard = spec["shard"]
    for k in range(NCORES):
        r = res.results[k]["out"]
        b = cores[k]["bounds"]
        for j in range(npanel):
            a, e = int(b[j]), int(b[j + 1])
            if e > a:
                out[k * shard + a:k * shard + e] = r[j * P:j * P + (e - a)]
    return out


# revision 13
# speedup vs baseline: 1.0829x; 1.0829x over previous
"""2-layer GCN (GCNConv x2, relu) on 8 Trainium2 NeuronCores.

Strategy (dest-shard, aggregate in 128-dim space):
  out1 = relu((A x) @ W1 + b1)            [A@(x@W1) == (A@x)@W1]
  zg   = out1 @ W2, allgathered
  out2 = relu(A zg + b2)
A = Dc(Mw + I)Dr; the full norm dinv[src]*w*dinv[dst] is folded into the
one-hot edge weights on host (dinv computed host-side from edge_weight).

Layer 1 needs no device gather: the host ships x rows pre-arranged in
edge-chunk order (xe[slot, chunk, feat]); chunks stream sequentially.
Layer 2 gathers zg rows by edge via gpsimd dma_gather (int16 idx, source
split in 2 halves < 32768 rows). Self-loop terms in layer 2 avoid the
gather: they read the local zg_shard slab sequentially and use a
host-built diagonal one-hot.
Per chunk: one-hot oh[e,d] = (iota==lc_e)*w_e via one DVE tensor_scalar,
matmul-accumulate into the dest panel's PSUM tile (L1 feat-major,
L2 dest-major).
"""
import sys
import numpy as np

sys.path.insert(0, "/opt/trn_rl_repo")

import concourse.bass as bass  # noqa: F401
import concourse.bacc as bacc
import concourse.mybir as mybir
import concourse.tile as tile
from concourse.bass_utils import run_bass_kernel_spmd
from concourse.masks import make_identity

P = 128
NCORES = 8
G = 32  # chunks per wave (both load and gather waves)

F32 = mybir.dt.float32
F16 = mybir.dt.float16
I16 = mybir.dt.int16
MSG_DT = F16


# ---------------------------------------------------------------- CPU prep


def _pack_idx(idx_flat):
    """int16 indices -> [128, ceil(n/16)] wrapped + 8x replicated layout."""
    n = len(idx_flat)
    n16 = -(-n // 16)
    buf = np.zeros(16 * n16, dtype=np.int16)
    buf[:n] = idx_flat
    blk = buf.reshape(n16, 16).T  # idx j at [j%16, j//16]
    return np.tile(blk, (8, 1)).copy()


def preprocess(x, edge_index, edge_weight, n):
    """Per-core inputs + universal chunk grids.

    Dests grouped into panels of <=128 CONSECUTIVE dests, boundaries per
    core; per-panel chunk counts are universal (max over cores).
    """
    row = np.asarray(edge_index[0], dtype=np.int64)
    col = np.asarray(edge_index[1], dtype=np.int64)
    w = np.asarray(edge_weight, dtype=np.float32)
    shard = n // NCORES
    half = (n + 1) // 2

    # host-side gcn_norm (f64): deg includes self loop weight 1
    deg = np.ones(n, np.float64)
    np.add.at(deg, col, w.astype(np.float64))
    dinv = 1.0 / np.sqrt(deg)
    norm = (dinv[row] * w * dinv[col]).astype(np.float32)
    selfw = (dinv * dinv).astype(np.float32)

    core_of = col // shard
    ind_lo = np.bincount(col[row < half], minlength=n)
    ind_hi = np.bincount(col[row >= half], minlength=n)

    # ---- panel boundaries per core (greedy fill, caps per L2 stream)
    CAP = 7 * P
    blist = []
    for k in range(NCORES):
        lo_c = ind_lo[k * shard:(k + 1) * shard]
        hi_c = ind_hi[k * shard:(k + 1) * shard]
        b = [0]
        cl = ch = cd = 0
        for ld in range(shard):
            if cd == P or cl + lo_c[ld] > CAP or ch + hi_c[ld] > CAP:
                b.append(ld)
                cl = ch = cd = 0
            cl += lo_c[ld]
            ch += hi_c[ld]
            cd += 1
        b.append(shard)
        blist.append(b)
    npanel = max(len(b) - 1 for b in blist)
    bounds = np.zeros((NCORES, npanel + 1), np.int64)
    for k in range(NCORES):
        b = blist[k]
        while len(b) < npanel + 1:
            b.append(shard)
        bounds[k] = b

    # ---- per (core, panel) edge counts -> universal grids
    c1 = np.zeros((NCORES, npanel), np.int64)   # L1: edges + selfs
    clo = np.zeros((NCORES, npanel), np.int64)  # L2 lo (no selfs)
    chi = np.zeros((NCORES, npanel), np.int64)
    ind_all = ind_lo + ind_hi
    for k in range(NCORES):
        for j in range(npanel):
            a, b2 = bounds[k, j], bounds[k, j + 1]
            sl = slice(k * shard + a, k * shard + b2)
            c1[k, j] = ind_all[sl].sum() + (b2 - a)
            clo[k, j] = ind_lo[sl].sum()
            chi[k, j] = ind_hi[sl].sum()
    k1 = np.maximum(1, -(-c1.max(axis=0) // P))
    klo = -(-clo.max(axis=0) // P)
    khi = -(-chi.max(axis=0) // P)
    nch1 = int(k1.sum())
    nlo, nhi = int(klo.sum()), int(khi.sum())
    nch2 = npanel + nlo + nhi  # + one self chunk per panel

    chunks1 = []
    for j in range(npanel):
        for i in range(int(k1[j])):
            chunks1.append(dict(panel=j, first=(i == 0),
                                last=(i == int(k1[j]) - 1)))
    chunks2 = []
    lo_pos = hi_pos = 0
    for j in range(npanel):
        nj = 1 + int(klo[j]) + int(khi[j])
        chunks2.append(dict(stream=2, pos=j, panel=j, first=True,
                            last=(nj == 1)))
        ci = 1
        for i in range(int(klo[j])):
            ci += 1
            chunks2.append(dict(stream=0, pos=lo_pos, panel=j, first=False,
                                last=(ci == nj)))
            lo_pos += 1
        for i in range(int(khi[j])):
            ci += 1
            chunks2.append(dict(stream=1, pos=hi_pos, panel=j, first=False,
                                last=(ci == nj)))
            hi_pos += 1

    # ---- AG position map: node -> row in padded zg_full
    agpos = np.zeros(n, np.int64)
    for k in range(NCORES):
        for j in range(npanel):
            a, b2 = bounds[k, j], bounds[k, j + 1]
            if b2 > a:
                agpos[k * shard + a:k * shard + b2] = \
                    k * npanel * P + j * P + np.arange(b2 - a)
    h2 = int(agpos[half - 1]) + 1  # L2 lo/hi split row in zg_full
    assert h2 < 32768 and (NCORES * npanel * P - h2) < 32768

    # ---- per-core slot data
    cores = []
    for k in range(NCORES):
        m = core_of == k
        r_k, c_k, w_k = row[m], col[m], norm[m]
        ld = c_k - k * shard
        panel = np.searchsorted(bounds[k], ld, side="right") - 1
        q = ld - bounds[k][panel]
        hi = (r_k >= half).astype(np.int64)
        order = np.lexsort((hi, panel))
        r_k, w_k, panel, hi, q = (r_k[order], w_k[order], panel[order],
                                  hi[order], q[order])

        # L1 slots: per panel, real edges then selfs
        src1 = np.zeros(nch1 * P, np.int64)
        lc1 = np.zeros((P, nch1), np.float32)
        w1v = np.zeros((P, nch1), np.float32)
        off1 = np.r_[0, np.cumsum(k1)]
        pstart = np.searchsorted(panel, np.arange(npanel))
        pend = np.searchsorted(panel, np.arange(npanel), side="right")
        for j in range(npanel):
            a, b2 = bounds[k, j], bounds[k, j + 1]
            width = int(b2 - a)
            ss, se = int(pstart[j]), int(pend[j])
            srcs = np.concatenate([
                r_k[ss:se],
                np.arange(k * shard + a, k * shard + b2)])
            lcs = np.concatenate([q[ss:se], np.arange(width)])
            wvs = np.concatenate([w_k[ss:se],
                                  selfw[k * shard + a:k * shard + b2]])
            base = int(off1[j]) * P
            ne = len(srcs)
            src1[base:base + ne] = srcs
            fl = lcs.astype(np.float32)
            fv = wvs.astype(np.float32)
            cix = np.arange(ne) // P + int(off1[j])
            eix = np.arange(ne) % P
            lc1[eix, cix] = fl
            w1v[eix, cix] = fv

        # L2 slots: lo/hi streams by (panel, hi) groups
        key = panel * 2 + hi
        cnt = np.bincount(key, minlength=npanel * 2)
        goff = np.r_[0, np.cumsum(cnt)]
        idx_lo = np.zeros(nlo * P, np.int64)
        idx_hi = np.zeros(nhi * P, np.int64)
        lc2 = np.zeros((P, nch2), np.float32)
        w2v = np.zeros((P, nch2), np.float32)
        ag_r = agpos[r_k]
        win = np.zeros(npanel * 2, np.int64)
        lo_i = hi_i = 0
        for ci, c in enumerate(chunks2):
            st, j = c["stream"], c["panel"]
            if st == 2:
                continue
            g2 = j * 2 + st
            a = goff[g2] + win[g2] * P
            b2 = min(goff[g2] + win[g2] * P + P, goff[g2 + 1])
            win[g2] += 1
            m2 = max(0, int(b2 - a))
            if m2 > 0:
                if st == 0:
                    pos = c["pos"] * P
                    idx_lo[pos:pos + m2] = ag_r[a:b2]
                else:
                    pos = c["pos"] * P
                    idx_hi[pos:pos + m2] = ag_r[a:b2] - h2
                lc2[:m2, ci] = q[a:b2]
                w2v[:m2, ci] = w_k[a:b2]

        # self diagonal one-hot [P, npanel, P]
        diag2 = np.zeros((P, npanel, P), np.float16)
        for j in range(npanel):
            a, b2 = bounds[k, j], bounds[k, j + 1]
            width = int(b2 - a)
            qq = np.arange(width)
            diag2[qq, j, qq] = selfw[k * shard + a:k * shard + b2]

        cores.append(dict(
            src1=src1, lc1=lc1, w1v=w1v, lc2=lc2, w2v=w2v,
            idx2_lo=_pack_idx(idx_lo.astype(np.int16)),
            idx2_hi=_pack_idx(idx_hi.astype(np.int16)),
            diag2=diag2, bounds=bounds[k].copy()))

    spec = dict(n=n, shard=shard, npanel=npanel, half=half, h2=h2,
                chunks1=chunks1, chunks2=chunks2, nch1=nch1, nch2=nch2,
                nlo=nlo, nhi=nhi)
    return spec, cores


# ---------------------------------------------------------------- program


def build_program(spec, din, dhid, dout):
    npanel, shard = spec["npanel"], spec["shard"]
    h2 = spec["h2"]
    nrows_pad = npanel * P
    nfull = NCORES * nrows_pad
    chunks1, chunks2 = spec["chunks1"], spec["chunks2"]
    nch1, nch2 = spec["nch1"], spec["nch2"]
    nlo, nhi = spec["nlo"], spec["nhi"]
    assert din == P and dout == P and dhid == 2 * P

    nc = bacc.Bacc("TRN2", target_bir_lowering=False, debug=False,
                   num_devices=NCORES)
    dt = F32
    xe_d = nc.dram_tensor("xe", [P, nch1, din], MSG_DT, kind="ExternalInput")
    w1_d = nc.dram_tensor("w1", [din, dhid], MSG_DT, kind="ExternalInput")
    w2_d = nc.dram_tensor("w2", [dhid, dout], MSG_DT, kind="ExternalInput")
    lc1_d = nc.dram_tensor("lc1", [P, nch1], MSG_DT, kind="ExternalInput")
    w1v_d = nc.dram_tensor("w1v", [P, nch1], MSG_DT, kind="ExternalInput")
    lc2_d = nc.dram_tensor("lc2", [P, nch2], MSG_DT, kind="ExternalInput")
    w2v_d = nc.dram_tensor("w2v", [P, nch2], MSG_DT, kind="ExternalInput")
    i2lo_d = nc.dram_tensor("idx2_lo", [P, max(nlo, 1) * 8], I16,
                            kind="ExternalInput")
    i2hi_d = nc.dram_tensor("idx2_hi", [P, max(nhi, 1) * 8], I16,
                            kind="ExternalInput")
    diag2_d = nc.dram_tensor("diag2", [P, npanel, P], MSG_DT,
                             kind="ExternalInput")
    iota_d = nc.dram_tensor("iota", [P, G, P], MSG_DT, kind="ExternalInput")
    out_d = nc.dram_tensor("out", [nrows_pad, dout], dt,
                           kind="ExternalOutput")

    with tile.TileContext(nc) as tc:
        with (
            tc.tile_pool(name="const", bufs=1) as cpool,
            tc.tile_pool(name="dram", bufs=1, space="DRAM") as dram,
        ):
            zg_shard = dram.tile([nrows_pad, dout], MSG_DT)
            zg_full = dram.tile([nfull, dout], MSG_DT)

            iota_sb = cpool.tile([P, G, P], MSG_DT, tag="iota")
            nc.sync.dma_start(out=iota_sb[:], in_=iota_d[:])
            ident = cpool.tile([P, P], MSG_DT, tag="ident")
            make_identity(nc, ident[:])
            w1_sb = cpool.tile([din, dhid], MSG_DT, tag="w1")
            nc.sync.dma_start(out=w1_sb[:], in_=w1_d[:])
            w2a_sb = cpool.tile([P, dout], MSG_DT, tag="w2a")
            nc.sync.dma_start(out=w2a_sb[:], in_=w2_d[0:P, :])
            w2b_sb = cpool.tile([P, dout], MSG_DT, tag="w2b")
            nc.sync.dma_start(out=w2b_sb[:], in_=w2_d[P:2 * P, :])
            lc1_sb = cpool.tile([P, nch1], MSG_DT, tag="lc1")
            nc.sync.dma_start(out=lc1_sb[:], in_=lc1_d[:])
            w1v_sb = cpool.tile([P, nch1], MSG_DT, tag="w1v")
            nc.sync.dma_start(out=w1v_sb[:], in_=w1v_d[:])
            lc2_sb = cpool.tile([P, nch2], MSG_DT, tag="lc2")
            nc.sync.dma_start(out=lc2_sb[:], in_=lc2_d[:])
            w2v_sb = cpool.tile([P, nch2], MSG_DT, tag="w2v")
            nc.sync.dma_start(out=w2v_sb[:], in_=w2v_d[:])

            def build_oh_wave(ohp, lc_sb, wv_sb, w0, gsz):
                """One-hot wave: oh[e,c,d] = (iota==lc[e,c]) * w[e,c]."""
                ohw = ohp.tile([P, G, P], MSG_DT, tag="ohw")
                nc.vector.tensor_tensor(
                    out=ohw[:, :gsz, :], in0=iota_sb[:, :gsz, :],
                    in1=lc_sb[:, w0:w0 + gsz, None].broadcast_to(
                        [P, gsz, P]),
                    op=mybir.AluOpType.is_equal)
                nc.vector.tensor_tensor(
                    out=ohw[:, :gsz, :], in0=ohw[:, :gsz, :],
                    in1=wv_sb[:, w0:w0 + gsz, None].broadcast_to(
                        [P, gsz, P]),
                    op=mybir.AluOpType.mult)
                return ohw
            i2lo_sb = cpool.tile([P, max(nlo, 1) * 8], I16, tag="i2lo")
            nc.sync.dma_start(out=i2lo_sb[:], in_=i2lo_d[:])
            i2hi_sb = cpool.tile([P, max(nhi, 1) * 8], I16, tag="i2hi")
            nc.sync.dma_start(out=i2hi_sb[:], in_=i2hi_d[:])
            diag2_sb = cpool.tile([P, npanel, P], MSG_DT, tag="diag2")
            nc.sync.dma_start(out=diag2_sb[:], in_=diag2_d[:])

            # -------- layer 1: stream xe chunks, one-hot matmul ----------
            def l1_panel(j, psum, epp, sbp):
                aggT = sbp.tile([P, P], MSG_DT, tag="aggT")
                nc.vector.tensor_copy(out=aggT[:], in_=psum[:])
                h1p = epp.tile([P, dhid], F32, space="PSUM", tag="h1p")
                nc.tensor.matmul(out=h1p[:], lhsT=aggT[:], rhs=w1_sb[:],
                                 start=True, stop=True)
                h1 = sbp.tile([P, dhid], MSG_DT, tag="h1")
                nc.vector.tensor_scalar(out=h1[:], in0=h1p[:],
                                        scalar1=0.0, scalar2=None,
                                        op0=mybir.AluOpType.max)
                tp0 = epp.tile([P, P], MSG_DT, space="PSUM", tag="tp0")
                nc.tensor.transpose(out=tp0[:], in_=h1[:, 0:P],
                                    identity=ident[:])
                tp1 = epp.tile([P, P], MSG_DT, space="PSUM", tag="tp1")
                nc.tensor.transpose(out=tp1[:], in_=h1[:, P:2 * P],
                                    identity=ident[:])
                h1t0 = sbp.tile([P, P], MSG_DT, tag="h1t0")
                nc.vector.tensor_copy(out=h1t0[:], in_=tp0[:])
                h1t1 = sbp.tile([P, P], MSG_DT, tag="h1t1")
                nc.vector.tensor_copy(out=h1t1[:], in_=tp1[:])
                zp = epp.tile([P, dout], F32, space="PSUM", tag="zp")
                nc.tensor.matmul(out=zp[:], lhsT=h1t0[:], rhs=w2a_sb[:],
                                 start=True, stop=False)
                nc.tensor.matmul(out=zp[:], lhsT=h1t1[:], rhs=w2b_sb[:],
                                 start=False, stop=True)
                zg = sbp.tile([P, dout], MSG_DT, tag="zg")
                nc.vector.tensor_copy(out=zg[:], in_=zp[:])
                nc.sync.dma_start(out=zg_shard[j * P:(j + 1) * P, :],
                                  in_=zg[:])

            with (
                tc.tile_pool(name="xw", bufs=3) as xwp,
                tc.tile_pool(name="oh1", bufs=3) as ohp1,
                tc.tile_pool(name="agg1", bufs=2, space="PSUM") as aggp1,
                tc.tile_pool(name="ep1", bufs=1, space="PSUM") as epp1,
                tc.tile_pool(name="sb1", bufs=3) as sbp1,
            ):
                wave_t = None
                ohw = None
                psum = None
                for ci, c in enumerate(chunks1):
                    wv, slot = divmod(ci, G)
                    if slot == 0:
                        gsz = min(G, nch1 - wv * G)
                        wave_t = xwp.tile([P, G, P], MSG_DT, tag="xw")
                        nc.sync.dma_start(
                            out=wave_t[:, :gsz, :],
                            in_=xe_d[:, wv * G:wv * G + gsz, :])
                        ohw = build_oh_wave(ohp1, lc1_sb, w1v_sb,
                                            wv * G, gsz)
                    gt = wave_t[:, slot, :]
                    oh = ohw[:, slot, :]
                    if c["first"]:
                        psum = aggp1.tile([P, P], F32, space="PSUM",
                                          tag="agg")
                    # feat-major: psum[f,d] += gt.T @ oh
                    nc.tensor.matmul(out=psum[:], lhsT=gt, rhs=oh,
                                     start=c["first"], stop=c["last"])
                    if c["last"]:
                        l1_panel(c["panel"], psum, epp1, sbp1)

            # -------- allgather ------------------------------------------
            nc.gpsimd.collective_compute(
                "AllGather", mybir.AluOpType.bypass,
                replica_groups=[list(range(NCORES))],
                ins=[zg_shard.opt()], outs=[zg_full.opt()])

            # -------- layer 2: gather zg, one-hot matmul -----------------
            def l2_panel(j, psum, sbp):
                o = sbp.tile([P, dout], dt, tag="o2")
                nc.vector.tensor_scalar(out=o[:], in0=psum[:],
                                        scalar1=0.0, scalar2=None,
                                        op0=mybir.AluOpType.max)
                nc.sync.dma_start(out=out_d[j * P:(j + 1) * P, :],
                                  in_=o[:])

            with (
                tc.tile_pool(name="glo", bufs=3) as glo,
                tc.tile_pool(name="ghi", bufs=3) as ghi,
                tc.tile_pool(name="gse", bufs=3) as gse,
                tc.tile_pool(name="oh2", bufs=3) as ohp2,
                tc.tile_pool(name="agg2", bufs=4, space="PSUM") as aggp2,
                tc.tile_pool(name="sb2", bufs=3) as sbp2,
            ):
                wave_t = [None, None]
                ohw = None
                psum = None
                for ci, c in enumerate(chunks2):
                    st, pos, j = c["stream"], c["pos"], c["panel"]
                    wv2, slot2 = divmod(ci, G)
                    if slot2 == 0:
                        gsz2 = min(G, nch2 - wv2 * G)
                        ohw = build_oh_wave(ohp2, lc2_sb, w2v_sb,
                                            wv2 * G, gsz2)
                    if st == 2:
                        gt_t = gse.tile([P, P], MSG_DT, tag="gse")
                        nc.sync.dma_start(
                            out=gt_t[:],
                            in_=zg_shard[j * P:(j + 1) * P, :])
                        gt = gt_t[:]
                        oh = diag2_sb[:, j, :]
                    else:
                        wv, slot = divmod(pos, G)
                        if slot == 0:
                            pool = glo if st == 0 else ghi
                            idx_sb = i2lo_sb if st == 0 else i2hi_sb
                            nw = nlo if st == 0 else nhi
                            src = (zg_full[0:h2, :] if st == 0
                                   else zg_full[h2:nfull, :])
                            gsz = min(G, nw - wv * G)
                            t = pool.tile([P, G, P], MSG_DT, tag="gw")
                            nc.gpsimd.dma_gather(
                                out_ap=t[:, :gsz, :], in_ap=src,
                                idxs_ap=idx_sb[:, wv * G * 8:
                                               wv * G * 8 + gsz * 8],
                                num_idxs=gsz * P, num_idxs_reg=gsz * P,
                                elem_size=P, single_packet=False)
                            wave_t[st] = t
                        gt = wave_t[st][:, slot, :]
                        oh = ohw[:, slot2, :]
                    if c["first"]:
                        psum = aggp2.tile([P, P], F32, space="PSUM",
                                          tag="agg")
                    # dest-major: psum[d,f] += oh.T @ gt
                    nc.tensor.matmul(out=psum[:], lhsT=oh, rhs=gt,
                                     start=c["first"], stop=c["last"])
                    if c["last"]:
                        l2_panel(j, psum, sbp2)

    nc.compile()
    return nc


# ---------------------------------------------------------------- kernel


def make_in_maps(spec, cores, x, W1, W2):
    n = spec["n"]
    nch1 = spec["nch1"]
    x32 = np.asarray(x, dtype=np.float32)
    W1m = np.asarray(W1, dtype=np.float32).astype(np.float16)
    W2m = np.asarray(W2, dtype=np.float32).astype(np.float16)
    iota_np = np.broadcast_to(
        np.arange(P, dtype=np.float16), (P, G, P)).copy()
    in_maps = []
    for k in range(NCORES):
        c = cores[k]
        xe = x32[c["src1"]].astype(np.float16)      # [nch1*P, din]
        mask = c["w1v"].T.reshape(-1) != 0           # zero padded slots
        xe[~mask] = 0
        xe = xe.reshape(nch1, P, -1).transpose(1, 0, 2).copy()
        in_maps.append(dict(
            xe=xe, w1=W1m, w2=W2m,
            lc1=c["lc1"].astype(np.float16),
            w1v=c["w1v"].astype(np.float16),
            lc2=c["lc2"].astype(np.float16),
            w2v=c["w2v"].astype(np.float16),
            idx2_lo=c["idx2_lo"], idx2_hi=c["idx2_hi"],
            diag2=c["diag2"], iota=iota_np))
    return in_maps


def kernel(x, edge_index, edge_weight, W1, b1, W2, b2):
    x = np.asarray(x, dtype=np.float32)
    W1 = np.asarray(W1, dtype=np.float32)
    W2 = np.asarray(W2, dtype=np.float32)
    n, din = x.shape
    dhid, dout = W1.shape[1], W2.shape[1]
    assert not np.any(np.asarray(b1)) and not np.any(np.asarray(b2))

    spec, cores = preprocess(x, edge_index, edge_weight, n)
    nc = build_program(spec, din, dhid, dout)
    in_maps = make_in_maps(spec, cores, x, W1, W2)

    res = run_bass_kernel_spmd(nc, in_maps, core_ids=list(range(NCORES)))
    out = np.empty((n, dout), dtype=np.float32)
    npanel = spec["npanel"]
    shard = spec["shard"]
    for k in range(NCORES):
        r = res.results[k]["out"]
        b = cores[k]["bounds"]
        for j in range(npanel):
            a, e = int(b[j]), int(b[j + 1])
            if e > a:
                out[k * shard + a:k * shard + e] = r[j * P:j * P + (e - a)]
    return out
